# revision 2
# baseline (speedup 1.0000x reference)
"""Trainium2 Bass kernel v2 for nn_ChordHMM: HMM forward-algorithm NLL.

Probability-space recursion p' = w_t (*) (A^T p) over T=4000 frames, S=170
states padded to SP=192.  128 time-segments (L=32 real steps + W=8 warmup;
the initial condition is forgotten geometrically, host-validated 5e-6 rel),
16 per core, fused as 4 groups x 4 chains so each matmul streams N=128
columns (4 chains x 32 songs).  Groups pair into 2 independent superchains
that interleave on the engines to hide PE->evac->PE latency.

Per superchain per step (bf16 matmuls, fp32 PSUM):
  8 MMs into a 3-bank psum tile: bank0 = even-group states 0:128, bank1 =
  odd-group states 0:128, bank2 = packed high states 128:192 (even rows
  0:64 via tile_position (0,0), odd rows 64:128 via (0,64)).
  DVE: direct TT bank0 x w -> p' (1x from PSUM).
  ACT: copy banks 1:3 -> bf16 staging;  DVE: bf16 TT staging x w -> p' (2x).

Host side: w = exp(0.8 x + C) slab in bf16 (layout mirrors the psum banks),
exact log-sum-exp correction, init vector, fp64 stitching.  Segment starts
t0(s) = 1 + floor(s*(T-1-L)/(NSEG-1)); duplicated boundary frames (0 or 1
per segment) are cancelled with the after-first-step colsum csD.
"""

import numpy as np
import ml_dtypes

import concourse.bass as bass
import concourse.bacc as bacc
import concourse.tile as tile
from concourse import mybir
from concourse.bass_utils import run_bass_kernel_spmd

F32 = mybir.dt.float32
BF16 = mybir.dt.bfloat16
NP_BF16 = ml_dtypes.bfloat16

S, B, T = 170, 32, 4000
SP = 192                     # padded state count
TEMP, EW = 0.5, 0.8
CSH = -0.32                  # drift-zeroing shift
NCORE = 8
NSEG = 192
L = 21                       # real steps per segment
W = 6                        # warmup steps
STEPS = L + W                # 27
N = 128                      # columns per group (4 chains x 32 songs)
NSC = 3                      # superchains per core
# DMA chunk boundaries over steps: fine-grained at the start so step 0 is not
# gated on a large transfer, coarse afterwards
_CB = [0, 1, 3, 6, 11, 16, 21, 27]
CHUNKS = [(a, b) for a, b in zip(_CB[:-1], _CB[1:])]
NCHUNK = len(CHUNKS)


def _seg_t0(s):
    return 1 + (s * (T - 1 - L)) // (NSEG - 1)


def build_bass(bench_repeat=None):
    nc = bacc.Bacc(None)
    emt = nc.dram_tensor("emt", [128, STEPS * NSC * 3 * N], BF16, kind="ExternalInput")
    trans = nc.dram_tensor("trans", [128, 2 * SP], BF16, kind="ExternalInput")
    initd = nc.dram_tensor("init", [128, NSC * 3 * N], BF16, kind="ExternalInput")
    maskd = nc.dram_tensor("mask", [128, NSC * 3 * N], BF16, kind="ExternalInput")
    sums = nc.dram_tensor("sums", [1, 3072], F32, kind="ExternalOutput")

    from contextlib import ExitStack

    with tile.TileContext(nc) as tc, ExitStack() as ctx:
        singles = ctx.enter_context(tc.tile_pool(name="singles", bufs=1))
        stgpool = ctx.enter_context(tc.tile_pool(name="stg", bufs=4))
        psp = [ctx.enter_context(tc.tile_pool(name=f"ps{k}", bufs=2, space="PSUM"))
               for k in range(NSC)]
        cspool = ctx.enter_context(tc.tile_pool(name="cs", bufs=1, space="PSUM"))

        # transition blocks (lhsT = A-block [K, M]; out = A_blk^T @ p).
        # High-state blocks are duplicated at partition bases 0 and 64 because
        # walrus requires Fmap and Weight to share the SBUF start partition.
        tA0 = singles.tile([128, SP], BF16, tag="tA0")
        tAh = singles.tile([128, SP], BF16, tag="tAh")
        nc.sync.dma_start(out=tA0, in_=trans[:, 0:SP])
        nc.sync.dma_start(out=tAh, in_=trans[:, SP:2 * SP])
        A00 = tA0[:, 0:128]
        A01 = tA0[:, 128:192]
        A10a, A10b = tAh[0:64, 0:128], tAh[64:128, 0:128]
        A11a, A11b = tAh[0:64, 128:192], tAh[64:128, 128:192]

        ivt = singles.tile([128, NSC, 3, N], BF16, tag="ivt")
        mkt = singles.tile([128, NSC, 3, N], BF16, tag="mkt")
        nc.sync.dma_start(out=ivt, in_=initd[:, :])
        nc.sync.dma_start(out=mkt, in_=maskd[:, :])

        ones128 = singles.tile([128, 1], BF16, tag="ones128")
        nc.vector.memset(ones128, 1.0)

        sums_sb = singles.tile([1, 3072], F32, tag="sums_sb")
        nc.vector.memset(sums_sb, 0.0)

        # w slab chunks: [128, chunk_steps, 2 superchains, 3 banks, N]
        wch = [singles.tile([128, b - a, NSC, 3, N], BF16,
                            tag=f"wch{k}", name=f"wch{k}")
               for k, (a, b) in enumerate(CHUNKS)]

        # ping-pong p tiles per superchain
        pp = [[singles.tile([128, 3, N], BF16, tag=f"pp{k}_{i}", name=f"pp{k}_{i}")
               for i in range(2)] for k in range(NSC)]

        def mms(ps, src):
            h0a, h0b = src[:, 0, :], src[:, 1, :]
            h1a, h1b = src[0:64, 2, :], src[64:128, 2, :]
            # even group (banks 0, 2-low)
            nc.tensor.matmul(ps[:, 0, :], A00, h0a, start=True, stop=False,
                             tile_position=(0, 0))
            nc.tensor.matmul(ps[:, 0, :], A10a, h1a, start=False, stop=True,
                             tile_position=(0, 0), skip_group_check=True)
            nc.tensor.matmul(ps[0:64, 2, :], A01, h0a, start=True, stop=False,
                             tile_position=(0, 0), skip_group_check=True)
            nc.tensor.matmul(ps[0:64, 2, :], A11a, h1a, start=False, stop=True,
                             tile_position=(0, 0), skip_group_check=True)
            # odd group (banks 1, 2-high)
            nc.tensor.matmul(ps[:, 1, :], A00, h0b, start=True, stop=False,
                             tile_position=(0, 0), skip_group_check=True)
            nc.tensor.matmul(ps[:, 1, :], A10b, h1b, start=False, stop=True,
                             tile_position=(64, 0), skip_group_check=True)
            nc.tensor.matmul(ps[64:128, 2, :], A01, h0b, start=True, stop=False,
                             tile_position=(0, 64), skip_group_check=True)
            nc.tensor.matmul(ps[64:128, 2, :], A11b, h1b, start=False, stop=True,
                             tile_position=(64, 64), skip_group_check=True)

        def step_mm(k, j):
            src = pp[k][j % 2]
            ps = psp[k].tile([128, 3, N], F32, tag="ps")
            mms(ps, src)
            return ps

        def step_evac(k, j, ps):
            dst = pp[k][(j + 1) % 2]
            ci = next(i for i, (a, b) in enumerate(CHUNKS) if a <= j < b)
            wj = wch[ci][:, j - CHUNKS[ci][0], k]
            if k == NSC - 1:
                # all-DVE path: single 1x TT straight from PSUM
                nc.vector.tensor_mul(dst, ps, wj)
            else:
                # ACT evacuates+casts all 3 banks, DVE multiplies at 2x
                stg = stgpool.tile([128, 3, N], BF16, tag="stg")
                nc.scalar.copy(stg, ps)
                nc.vector.tensor_mul(dst, stg, wj)

        def colsum(k, j, ev):
            # column sums of pp[k][j % 2] -> sums_sb[(ev*2+k)*256 : +256]
            p = pp[k][j % 2]
            cs = cspool.tile([1, 2, N], F32, tag="cs")
            nc.tensor.matmul(cs[:, 0, :], ones128, p[:, 0, :],
                             start=True, stop=False, tile_position=(0, 0))
            nc.tensor.matmul(cs[:, 0, :], ones128[0:64], p[0:64, 2, :],
                             start=False, stop=True, tile_position=(0, 0),
                             skip_group_check=True)
            nc.tensor.matmul(cs[:, 1, :], ones128, p[:, 1, :],
                             start=True, stop=False, tile_position=(0, 0),
                             skip_group_check=True)
            nc.tensor.matmul(cs[:, 1, :], ones128[64:128], p[64:128, 2, :],
                             start=False, stop=True, tile_position=(64, 0),
                             skip_group_check=True)
            slot = ev * 4 + k
            nc.scalar.copy(sums_sb[:, slot * 256:(slot + 1) * 256], cs)

        def emit_body():
            for c, (a, b) in enumerate(CHUNKS):
                nc.sync.dma_start(out=wch[c], in_=emt[:, a * NSC * 3 * N:b * NSC * 3 * N])
            for k in range(NSC):
                nc.vector.memset(pp[k][0], 1.0 / S)
            def pre_mm(k, j):
                if j == W:
                    P = pp[k][j % 2]
                    nc.vector.tensor_mul(P, P, mkt[:, k])
                    nc.vector.tensor_add(P, P, ivt[:, k])
                    colsum(k, j, 0)                   # csS
            def post_evac(k, j):
                if j == W:
                    colsum(k, j + 1, 1)               # csD (after first real step)
            for j in range(STEPS):
                pss = []
                for k in range(NSC):
                    pre_mm(k, j)
                    pss.append(step_mm(k, j))
                for k in range(NSC):
                    step_evac(k, j, pss[k])
                    post_evac(k, j)
            for k in range(NSC):
                colsum(k, STEPS, 2)                   # csE

        if bench_repeat is None:
            emit_body()
        else:
            with tc.For_i(0, bench_repeat, 1):
                emit_body()
        nc.sync.dma_start(out=sums[:, :], in_=sums_sb)

    nc.finalize()
    return nc


_NC_CACHE = None


def _get_nc():
    global _NC_CACHE
    if _NC_CACHE is None:
        _NC_CACHE = build_bass()
    return _NC_CACHE


def _log_softmax64(x, axis=-1):
    x = np.asarray(x, dtype=np.float64)
    m = x.max(axis=axis, keepdims=True)
    return x - m - np.log(np.sum(np.exp(x - m), axis=axis, keepdims=True))


def prepare_inputs(emissions, start_probs, raw_transitions):
    em = np.ascontiguousarray(np.asarray(emissions, dtype=np.float32))
    sp = np.asarray(start_probs, dtype=np.float32)
    rt = np.asarray(raw_transitions, dtype=np.float32)

    A = np.exp(_log_softmax64(rt / TEMP))                       # [S,S] fp64
    A192 = np.zeros((SP, SP), np.float64)
    A192[:S, :S] = A
    A192 = A192.astype(NP_BF16)
    transd = np.zeros((128, 2 * SP), NP_BF16)                   # blocks, hi x2
    transd[:, 0:SP] = A192[0:128]
    transd[0:64, SP:2 * SP] = A192[128:192]
    transd[64:128, SP:2 * SP] = A192[128:192]
    pstart = np.exp(_log_softmax64(sp))                         # [S] fp64

    # exact emission log-sum-exp correction (fp32 exp, fp64 reduce)
    z = np.exp(em, dtype=np.float32).sum(-1, dtype=np.float64)  # [B,T]
    lse_sum = np.log(z).sum(axis=1)                             # [B]

    # w slab [T, B, SP] bf16
    w = np.zeros((T, B, SP), NP_BF16)
    w[:, :, :S] = np.exp(EW * em.transpose(1, 0, 2) + CSH)

    x0 = em[:, 0, :].astype(np.float64)
    init0 = np.zeros((B, SP), np.float64)
    init0[:, :S] = pstart[None, :] * np.exp(EW * x0 + CSH)      # [B,SP]
    init0 = init0.astype(NP_BF16)

    in_maps = []
    for core in range(NCORE):
        emt = np.zeros((128, STEPS, NSC, 3, N), NP_BF16)
        init = np.zeros((128, NSC, 3, N), NP_BF16)
        mask = np.ones((128, NSC, 3, N), NP_BF16)
        for g in range(2 * NSC):
            k, half = g // 2, g % 2
            rows1 = slice(0, 64) if half == 0 else slice(64, 128)
            for c in range(4):
                seg = core * (8 * NSC) + g * 4 + c
                t0 = _seg_t0(seg)
                ts = np.clip(np.arange(t0 - W, t0 + L), 0, T - 1)
                blk = w[ts]                                     # [STEPS,B,SP]
                cols = slice(c * B, (c + 1) * B)
                emt[:, :, k, half, cols] = blk[:, :, 0:128].transpose(2, 0, 1)
                emt[rows1, :, k, 2, cols] = blk[:, :, 128:192].transpose(2, 0, 1)
                if seg == 0:
                    mask[:, k, half, cols] = 0.0
                    mask[rows1, k, 2, cols] = 0.0
                    init[:, k, half, cols] = init0[:, 0:128].T
                    init[rows1, k, 2, cols] = init0[:, 128:192].T
        in_maps.append({
            "emt": np.ascontiguousarray(emt.reshape(128, STEPS * NSC * 3 * N)),
            "trans": transd,
            "init": init.reshape(128, NSC * 3 * N),
            "mask": mask.reshape(128, NSC * 3 * N),
        })
    return in_maps, lse_sum


def stitch(results, lse_sum):
    csS = np.empty((NSEG, B))
    csD = np.empty((NSEG, B))
    csE = np.empty((NSEG, B))
    for core in range(NCORE):
        sm = np.asarray(results[core]["sums"], np.float64).reshape(3, 4, 2, 4, B)
        for g in range(2 * NSC):
            k, half = g // 2, g % 2
            for c in range(4):
                seg = core * (8 * NSC) + g * 4 + c
                csS[seg] = sm[0, k, half, c]
                csD[seg] = sm[1, k, half, c]
                csE[seg] = sm[2, k, half, c]
    llk = np.log(csE).sum(axis=0) - np.log(csS).sum(axis=0)
    llk += np.log(csS[0])
    for s in range(1, NSEG):
        dup = (_seg_t0(s - 1) + L) - _seg_t0(s)
        if dup > 0:
            assert dup == 1
            llk -= np.log(csD[s]) - np.log(csS[s])
    llk -= EW * lse_sum
    llk -= np.float64(T) * CSH
    return (-llk).astype(np.float32)


def kernel(emissions, start_probs, raw_transitions):
    nc = _get_nc()
    in_maps, lse_sum = prepare_inputs(emissions, start_probs, raw_transitions)
    res = run_bass_kernel_spmd(nc, in_maps, core_ids=list(range(NCORE)))
    return stitch(res.results, lse_sum)
